# revision 32
# baseline (speedup 1.0000x reference)
"""KAN layer (cubic B-spline, uniform grid) for 8 Trainium2 NeuronCores.

Approach: the six cubic B-spline basis functions basis_j(x) = N(1.5x+4.5-j)
are fixed smooth functions of the single scalar x.  They are represented
(to ~1e-3 weighted RMS, fitted offline against the exact Cox-de-Boor
recursion under the N(0,1) input distribution) in the span of seven cheap
feature maps:
    {1, silu(x), exp(-a_i (xc - m_i)^2) i=0..5},  xc = clamp(x, -3, 3)
The constant channel folds into the output bias; the remaining 7 features
feed a fp16 matmul with host-combined weights
    V[o,d,f] = sum_j scale_sp[o,d] * coef[o,d,j] * C[f,j],
so per token the kernel computes  y = feats @ V^T + bias  with contraction
K = 512*7 (vs 512*13 for a truncated-power-basis formulation).

On-chip per 512-token group: clamp + half-scale (vector), tanh + 6 exp
(scalar engine; silu(x) = (x/2)(1+tanh(x/2)) exactly, keeping every scalar
op in the exp/tanh activation-table set -> one ACT table load), 6 squared
distances (vector tensor_tensor, 2x mode), 29 matmuls per 128-token chunk.
The tensor engine is the bottleneck at ~24us/group.

Schedule details: all input DMAs are issued from the sync queue in an
explicit priority order (x blocks interleaved with weight chunks in
consumption order) because the per-engine DMA FIFOs drain in arrival
order.  The first 512 tokens are processed in small sub-groups with
token-stationary matmuls (psum [tok, o], vector-engine bias+evacuation)
so matmuls start ~10us in and keep the PE clock gate (HAM) open; the
remaining 1536 tokens use weight-stationary matmuls (psum [o, tok],
scalar-engine per-partition-bias evacuation, o-major y2 output that the
host transposes).  Each next range's clamp/tanh/silu head is hoisted
ahead of the current range's exp chain on the in-order scalar queue, and
PSUM evacuation for range r is emitted after range r+1's feature ops, so
neither the vector nor scalar queue ever stalls the next range's
features.  The last group runs o-block-outer so its evacuation overlaps
the matmul stream.

Data parallel over tokens: core c processes batch row c (2048 tokens).
"""

import numpy as np

import concourse.bass as bass
import concourse.mybir as mybir
import concourse.tile as tile
from concourse import bacc
from concourse.bass_utils import run_bass_kernel_spmd

F32 = mybir.dt.float32
F16 = mybir.dt.float16
ALU = mybir.AluOpType
AF = mybir.ActivationFunctionType

N_CORES = 8
D = 512          # in_features
O = 512          # out_features
TOK = 2048       # tokens per core
DT = D // 128    # d-tiles
NF = 7           # matmul features: silu + 6 gaussians

# offline fit of basis_j(u) ~ C[0,j] + C[1,j]*silu(x) + sum_i C[2+i,j]*g_i(u)
# with g_i = exp(-W_i (u - M_i)^2), u = 1.5x + 4.5 clamped to [0,9].
GAUSS_M = [2.0136448196366565, 3.0020084036623915, 4.0003483932813335,
           4.999588735443863, 5.997713911954367, 6.985305153374808]
GAUSS_W = [1.2619403072231266, 1.1976479467083618, 1.1876576172436166,
           1.1873248687663445, 1.1985693610575836, 1.2670653033973047]
FIT_C = np.array([
 [-0.014076312408546078, 0.0030093286289790723, -0.0015483720228174193, 0.001051762014029704, -0.0004897521315850754, -0.000887221022976455],
 [0.005876740626533684, -0.0015677343525573972, 0.0011509219211435948, -0.0011444952851484866, 0.0014410957474447201, -0.005142005556391447],
 [0.6883806333564002, -0.04083848226036093, 0.008060321332289084, -0.0037685760483973907, 0.0020933252562422002, -0.0007681411586616727],
 [-0.022738777814332027, 0.6876227273109307, -0.04291372848231555, 0.007127102690404596, -0.0022787476226903756, 0.0003672673217971428],
 [0.014126054568335171, -0.044924255427300344, 0.6933655085603457, -0.045792817865033045, 0.007933603676615806, -0.0015453651987665717],
 [0.006650677872101163, 0.005744884186467451, -0.044029610498844615, 0.6917600947426843, -0.04263220709635316, 0.005718618412934847],
 [0.0067666440225774855, -0.003981106455691672, 0.00836428843888494, -0.04422963468506508, 0.6892332271718113, -0.029107509213147646],
 [0.0036261271724442106, 0.0009189834250657185, -0.0028350790762476097, 0.007121213292172948, -0.03905988194390376, 0.6828562285636222],
], dtype=np.float64)   # rows: const, silu, g0..g5; cols: basis j

# gaussian centers in x-space and exp scales: g = exp(ESC_i * (xc - XC_i)^2)
XC = [(m - 4.5) / 1.5 for m in GAUSS_M]
ESC = [-2.25 * w for w in GAUSS_W]

# token ranges: first 512 tokens split fine so matmuls start early
RANGES = [(0, 128), (128, 128), (256, 256),
          (512, 512), (1024, 512), (1536, 512)]

_prog_cache = {}
last_results = None  # BassKernelResults of the most recent run (for test.py)


def _build_program():
    nc = bacc.Bacc("TRN2", target_bir_lowering=False, debug=False,
                   num_devices=N_CORES)
    # x pre-packed on host: xP[p, dt, t] = x[t, dt*128+p], fp16
    xP_d = nc.dram_tensor("xP", [128, DT, TOK], F16, kind="ExternalInput").ap()
    wf_d = nc.dram_tensor("wf", [128, NF * DT * O], F16, kind="ExternalInput").ap()
    br_d = nc.dram_tensor("biasrep", [128, O], F32, kind="ExternalInput").ap()
    bc_d = nc.dram_tensor("biascol", [128, DT], F32, kind="ExternalInput").ap()
    # first 512 tokens come out token-major, the rest o-major (host stitches)
    y_d = nc.dram_tensor("y", [512, O], F32, kind="ExternalOutput").ap()
    y2_d = nc.dram_tensor("y2", [O, TOK - 512], F32, kind="ExternalOutput").ap()

    with tile.TileContext(nc) as tc:
        with tc.tile_pool(name="const", bufs=1) as cpool, \
             tc.tile_pool(name="xg", bufs=2) as xpool, \
             tc.tile_pool(name="work", bufs=3) as wpool, \
             tc.tile_pool(name="feat", bufs=1) as fpool, \
             tc.tile_pool(name="outp", bufs=4) as opool, \
             tc.tile_pool(name="psum", bufs=6, space="PSUM") as pspool:

            wfall = cpool.tile([128, NF * DT * O], F16, name="wfall", tag="wfall")
            biasr = cpool.tile([128, O], F32, name="biasr_t", tag="biasr")

            # --- controlled DMA issue order on the sync queue ---
            # interleave group-0 sub-block x loads with weight chunks so the
            # first matmuls (silu feature, then g0..) have operands earliest.
            def xdma(eng, xg, t0, glen):
                eng.dma_start(
                    xg[:].rearrange("p (dt t) -> p dt t", dt=DT),
                    xP_d[:, :, t0:t0 + glen])

            # all x loads on the sync queue, interleaved with the weight
            # chunks in consumption order (the dep-free sync stream keeps
            # its emission order; DMA engines drain it FIFO per engine)
            xg_tiles = {}
            for t0, glen in RANGES:
                tag = f"xg_s{t0}" if glen <= 256 else f"xg_f{t0}"
                xg_tiles[t0] = xpool.tile([128, DT * glen], F16,
                                          name=f"xg{t0}", tag=tag, bufs=1)

            def wfdma(f):
                nc.sync.dma_start(
                    wfall[:, f * DT * O:(f + 1) * DT * O],
                    wf_d[:, f * DT * O:(f + 1) * DT * O])

            # priority order: x before the weight chunks that can wait;
            # the first feature's weights split per d-tile so the very
            # first matmuls are not gated on the whole 0.5MB chunk
            xdma(nc.sync, xg_tiles[0], 0, 128)
            for t_ in range(DT):
                nc.sync.dma_start(
                    wfall[:, t_ * O:(t_ + 1) * O],
                    wf_d[:, t_ * O:(t_ + 1) * O])
            xdma(nc.sync, xg_tiles[128], 128, 128)
            wfdma(1)
            xdma(nc.sync, xg_tiles[256], 256, 256)
            wfdma(2)
            xdma(nc.sync, xg_tiles[512], 512, 512)
            wfdma(3)
            wfdma(4)
            xdma(nc.sync, xg_tiles[1024], 1024, 512)
            wfdma(5)
            wfdma(6)
            xdma(nc.sync, xg_tiles[1536], 1536, 512)
            nc.sync.dma_start(biasr[:], br_d[:])
            biasc = cpool.tile([128, DT], F32, name="biasc_t", tag="biasc")
            nc.sync.dma_start(biasc[:], bc_d[:])
            wft = [wfall[:, i * O:(i + 1) * O] for i in range(NF * DT)]

            # HAM warmup: one accumulation group of full-width matmuls keeps
            # the PE continuously busy so the clock gate opens early; also
            # preload the exp/tanh ACT table set off the critical path.
            wdum = cpool.tile([128, O], F16, name="wdum", tag="wdum")
            nc.vector.memset(wdum[:], 0.0)
            wdum2 = cpool.tile([128, 16], F16, name="wdum2", tag="wdum2")
            nc.scalar.activation(wdum2[:], wdum[:, 0:16], AF.Tanh, scale=0.5)
            wps = pspool.tile([128, O], F32, name="wps", tag="wps", bufs=1)
            for _w in range(8):
                nc.tensor.matmul(wps[:], wdum[:, 0:128], wdum[:],
                                 start=(_w == 0), stop=(_w == 7))

            pending_sub = []    # (psum, row)  token-major head outputs
            pending_full = []   # (psum, ob, t0) o-major tail outputs

            def flush_sub(keep=0):
                while len(pending_sub) > keep:
                    ps, row = pending_sub.pop(0)
                    ot = opool.tile([128, O], F32, name="ot", tag="ot")
                    nc.vector.scalar_tensor_tensor(ot[:], ps[:], 1.0, biasr[:],
                                                   ALU.mult, ALU.add)
                    nc.gpsimd.dma_start(y_d[row:row + 128, :], ot[:])

            def flush_full():
                for ps, ob, t0g in pending_full:
                    ot2 = opool.tile([128, 512], F32, name="ot2", tag="ot2")
                    nc.scalar.activation(ot2[:], ps[:], AF.Identity,
                                         bias=biasc[:, ob:ob + 1])
                    nc.gpsimd.dma_start(
                        y2_d[ob * 128:(ob + 1) * 128,
                             t0g - 512:t0g - 512 + 512], ot2[:])
                pending_full.clear()

            heads = {}

            def emit_head(idx):
                t0, glen = RANGES[idx]
                FD = DT * glen
                xg = xg_tiles[t0]
                xc = wpool.tile([128, FD], F16, name=f"xc{t0}", tag="xc", bufs=2)
                nc.vector.tensor_scalar(xc[:], xg[:], 3.0, -3.0, ALU.min, ALU.max)
                xh = wpool.tile([128, FD], F16, name=f"xh{t0}", tag="xh", bufs=2)
                nc.vector.tensor_scalar(xh[:], xg[:], 0.5, 0.0, ALU.mult, ALU.add)
                th = wpool.tile([128, FD], F16, name=f"th{t0}", tag="th", bufs=2)
                nc.scalar.activation(th[:], xg[:], AF.Tanh, scale=0.5)
                th1 = wpool.tile([128, FD], F16, name=f"th1{t0}", tag="th1",
                                 bufs=2)
                nc.vector.tensor_scalar(th1[:], th[:], 1.0, 0.0,
                                        ALU.add, ALU.bypass)
                sil = fpool.tile([128, FD], F16, name=f"sil{t0}", tag="sil", bufs=2)
                # silu(x) = (x/2) * (1 + tanh(x/2))   (exact)
                nc.vector.tensor_mul(sil[:], th1[:], xh[:])
                heads[idx] = (xc, sil)

            for idx, (t0, glen) in enumerate(RANGES):
                FD = DT * glen
                if idx not in heads:
                    emit_head(idx)
                # hoist the next range's head (its tanh) ahead of this
                # range's exp chain on the in-order scalar queue
                if idx + 1 < len(RANGES) and (idx + 1) not in heads:
                    emit_head(idx + 1)
                xc, sil = heads[idx]
                feats = [sil]
                for i in range(6):
                    z = wpool.tile([128, FD], F16, name=f"z{t0}_{i}", tag="z",
                                   bufs=2)
                    nc.vector.tensor_scalar(z[:], xc[:], XC[i], 0.0,
                                            ALU.subtract, ALU.add)
                    s = wpool.tile([128, FD], F16, name=f"s{t0}_{i}", tag="s",
                                   bufs=2)
                    nc.vector.tensor_mul(s[:], z[:], z[:])
                    g = fpool.tile([128, FD], F16, name=f"g{t0}_{i}",
                                   tag=f"g{i}", bufs=2)
                    nc.scalar.activation(g[:], s[:], AF.Exp, scale=ESC[i])
                    feats.append(g)

                # previous block's PSUM evacuation comes after this block's
                # feature ops in the respective in-order engine queue
                n_mm = NF * DT
                if glen <= 256:
                    flush_sub(keep=1)
                    # token-stationary: lhsT = 128-token feature chunk,
                    # stream the 512-wide weight tiles
                    for c in range(glen // 128):
                        ps = pspool.tile([128, O], F32, name="ps", tag="ps",
                                         bufs=2)
                        i = 0
                        for f in feats:
                            for t_ in range(DT):
                                sl = f[:, t_ * glen + c * 128:
                                       t_ * glen + (c + 1) * 128]
                                nc.tensor.matmul(ps[:], sl, wft[i],
                                                 start=(i == 0),
                                                 stop=(i == n_mm - 1))
                                i += 1
                        pending_sub.append((ps, t0 + c * 128))
                else:
                    flush_sub()
                    flush_full()
                    # weight-stationary: stream 512 tokens per matmul,
                    # psum holds [128 o, 512 tok]; feature-outer order so
                    # each feature is consumed as soon as it is produced
                    pss = [pspool.tile([128, 512], F32, name="ps2",
                                       tag="ps2", bufs=5) for _ in range(4)]
                    if t0 == RANGES[-1][0]:
                        # last group: ob-outer so the four output blocks
                        # finish staggered and evacuation overlaps matmuls
                        # (all features are long since ready here)
                        for ob in range(4):
                            i = 0
                            for fi in range(NF):
                                for t_ in range(DT):
                                    nc.tensor.matmul(
                                        pss[ob][:],
                                        wft[fi * DT + t_][:, ob * 128:(ob + 1) * 128],
                                        feats[fi][:, t_ * glen:(t_ + 1) * glen],
                                        start=(i == 0), stop=(i == n_mm - 1))
                                    i += 1
                            pending_full.append((pss[ob], ob, t0))
                        flush_full()
                    else:
                        for fi in range(NF):
                            for t_ in range(DT):
                                for ob in range(4):
                                    nc.tensor.matmul(
                                        pss[ob][:],
                                        wft[fi * DT + t_][:, ob * 128:(ob + 1) * 128],
                                        feats[fi][:, t_ * glen:(t_ + 1) * glen],
                                        start=(fi == 0 and t_ == 0),
                                        stop=(fi == NF - 1 and t_ == DT - 1))
                        for ob in range(4):
                            pending_full.append((pss[ob], ob, t0))
            flush_sub()
            flush_full()
    nc.compile()
    return nc


def _host_tables(coef, scale_base, scale_sp, bias):
    W = (scale_sp[..., None] * coef).astype(np.float64)        # (O, D, 6)
    V = np.einsum("odj,fj->odf", W, FIT_C)                     # (O, D, 8)
    bias_eff = (bias.astype(np.float64) + V[:, :, 0].sum(1))   # const channel
    V = V[:, :, 1:]                                            # (O, D, NF)
    V[:, :, 0] += scale_base.astype(np.float64)                # silu channel
    # weight tiles: (f, dt) -> [128 d, O] fp16
    wfs = np.empty((NF * DT, 128, O), np.float16)
    for f in range(NF):
        for t in range(DT):
            wfs[f * DT + t] = V[:, t * 128:(t + 1) * 128, f].T
    wf = np.ascontiguousarray(wfs.transpose(1, 0, 2).reshape(128, NF * DT * O))
    biasrep = np.tile(bias_eff.astype(np.float32), (128, 1))
    biascol = np.ascontiguousarray(
        bias_eff.astype(np.float32).reshape(DT, 128).T)
    return wf, np.ascontiguousarray(biasrep), biascol


def kernel(x, coef, scale_base, scale_sp, bias, _trace=False):
    global last_results
    x = np.asarray(x, np.float32)
    coef = np.asarray(coef, np.float32)
    scale_base = np.asarray(scale_base, np.float32)
    scale_sp = np.asarray(scale_sp, np.float32)
    bias = np.asarray(bias, np.float32)
    B, S, Din = x.shape
    assert (B * S, Din) == (N_CORES * TOK, D), (x.shape,)

    if "nc" not in _prog_cache:
        _prog_cache["nc"] = _build_program()
    nc = _prog_cache["nc"]

    wf, biasrep, biascol = _host_tables(coef, scale_base, scale_sp, bias)
    xflat = x.reshape(N_CORES, TOK, D)
    in_maps = []
    for c in range(N_CORES):
        # xP[p, dt, t] = x[t, dt*128+p]
        xP = np.ascontiguousarray(
            xflat[c].T.reshape(DT, 128, TOK).transpose(1, 0, 2)
        ).astype(np.float16)
        in_maps.append({
            "xP": xP,
            "wf": wf, "biasrep": biasrep, "biascol": biascol,
        })
    kw = {}
    if _trace:
        kw.update(trace=True)
    last_results = run_bass_kernel_spmd(nc, in_maps,
                                        core_ids=list(range(N_CORES)), **kw)
    y = np.empty((N_CORES, TOK, O), np.float32)
    for c in range(N_CORES):
        r = last_results.results[c]
        y[c, :512] = r["y"]
        y[c, 512:] = r["y2"].T
    return y.reshape(B, S, O).astype(np.float32)


# revision 34
# speedup vs baseline: 1.0240x; 1.0240x over previous
"""KAN layer (cubic B-spline, uniform grid) for 8 Trainium2 NeuronCores.

Approach: the six cubic B-spline basis functions basis_j(x) = N(1.5x+4.5-j)
are fixed smooth functions of the single scalar x.  They are represented
(to ~1e-3 weighted RMS, fitted offline against the exact Cox-de-Boor
recursion under the N(0,1) input distribution) in the span of seven cheap
feature maps:
    {1, silu(x), exp(-a_i (xc - m_i)^2) i=0..5},  xc = clamp(x, -3, 3)
The constant channel folds into the output bias; the remaining 7 features
feed a fp16 matmul with host-combined weights
    V[o,d,f] = sum_j scale_sp[o,d] * coef[o,d,j] * C[f,j],
so per token the kernel computes  y = feats @ V^T + bias  with contraction
K = 512*7 (vs 512*13 for a truncated-power-basis formulation).

On-chip per 512-token group: clamp + half-scale (vector), tanh + 6 exp
(scalar engine; silu(x) = (x/2)(1+tanh(x/2)) exactly, keeping every scalar
op in the exp/tanh activation-table set -> one ACT table load), 6 squared
distances (vector tensor_tensor, 2x mode), 29 matmuls per 128-token chunk.
The tensor engine is the bottleneck at ~24us/group.

Schedule details: all input DMAs are issued from the sync queue in an
explicit priority order (x blocks interleaved with weight chunks in
consumption order) because the per-engine DMA FIFOs drain in arrival
order.  The first 512 tokens are processed in small sub-groups with
token-stationary matmuls (psum [tok, o], vector-engine bias+evacuation)
so matmuls start ~10us in and keep the PE clock gate (HAM) open; the
remaining 1536 tokens use weight-stationary matmuls (psum [o, tok],
scalar-engine per-partition-bias evacuation, o-major y2 output that the
host transposes).  Each next range's clamp/tanh/silu head is hoisted
ahead of the current range's exp chain on the in-order scalar queue, and
PSUM evacuation for range r is emitted after range r+1's feature ops, so
neither the vector nor scalar queue ever stalls the next range's
features.  The last group runs o-block-outer so its evacuation overlaps
the matmul stream.

Data parallel over tokens: core c processes batch row c (2048 tokens).
"""

import numpy as np

import concourse.bass as bass
import concourse.mybir as mybir
import concourse.tile as tile
from concourse import bacc
from concourse.bass_utils import run_bass_kernel_spmd

F32 = mybir.dt.float32
F16 = mybir.dt.float16
ALU = mybir.AluOpType
AF = mybir.ActivationFunctionType

N_CORES = 8
D = 512          # in_features
O = 512          # out_features
TOK = 2048       # tokens per core
DT = D // 128    # d-tiles
NF = 7           # matmul features: silu + 6 gaussians

# offline fit of basis_j(u) ~ C[0,j] + C[1,j]*silu(x) + sum_i C[2+i,j]*g_i(u)
# with g_i = exp(-W_i (u - M_i)^2), u = 1.5x + 4.5 clamped to [0,9].
GAUSS_M = [2.0136448196366565, 3.0020084036623915, 4.0003483932813335,
           4.999588735443863, 5.997713911954367, 6.985305153374808]
GAUSS_W = [1.2619403072231266, 1.1976479467083618, 1.1876576172436166,
           1.1873248687663445, 1.1985693610575836, 1.2670653033973047]
FIT_C = np.array([
 [-0.014076312408546078, 0.0030093286289790723, -0.0015483720228174193, 0.001051762014029704, -0.0004897521315850754, -0.000887221022976455],
 [0.005876740626533684, -0.0015677343525573972, 0.0011509219211435948, -0.0011444952851484866, 0.0014410957474447201, -0.005142005556391447],
 [0.6883806333564002, -0.04083848226036093, 0.008060321332289084, -0.0037685760483973907, 0.0020933252562422002, -0.0007681411586616727],
 [-0.022738777814332027, 0.6876227273109307, -0.04291372848231555, 0.007127102690404596, -0.0022787476226903756, 0.0003672673217971428],
 [0.014126054568335171, -0.044924255427300344, 0.6933655085603457, -0.045792817865033045, 0.007933603676615806, -0.0015453651987665717],
 [0.006650677872101163, 0.005744884186467451, -0.044029610498844615, 0.6917600947426843, -0.04263220709635316, 0.005718618412934847],
 [0.0067666440225774855, -0.003981106455691672, 0.00836428843888494, -0.04422963468506508, 0.6892332271718113, -0.029107509213147646],
 [0.0036261271724442106, 0.0009189834250657185, -0.0028350790762476097, 0.007121213292172948, -0.03905988194390376, 0.6828562285636222],
], dtype=np.float64)   # rows: const, silu, g0..g5; cols: basis j

# gaussian centers in x-space and exp scales: g = exp(ESC_i * (xc - XC_i)^2)
XC = [(m - 4.5) / 1.5 for m in GAUSS_M]
ESC = [-2.25 * w for w in GAUSS_W]

# token ranges: first 512 tokens split fine so matmuls start early
RANGES = [(0, 128), (128, 128), (256, 256),
          (512, 512), (1024, 512), (1536, 512)]

_prog_cache = {}
last_results = None  # BassKernelResults of the most recent run (for test.py)


def _build_program():
    nc = bacc.Bacc("TRN2", target_bir_lowering=False, debug=False,
                   num_devices=N_CORES)
    # x pre-packed on host: xP[p, dt, t] = x[t, dt*128+p], fp16
    xP_d = nc.dram_tensor("xP", [128, DT, TOK], F16, kind="ExternalInput").ap()
    wf_d = nc.dram_tensor("wf", [128, NF * DT * O], F16, kind="ExternalInput").ap()
    br_d = nc.dram_tensor("biasrep", [128, O], F32, kind="ExternalInput").ap()
    bc_d = nc.dram_tensor("biascol", [128, DT], F32, kind="ExternalInput").ap()
    # first 512 tokens come out token-major, the rest o-major (host stitches)
    y_d = nc.dram_tensor("y", [512, O], F32, kind="ExternalOutput").ap()
    y2_d = nc.dram_tensor("y2", [O, TOK - 512], F32, kind="ExternalOutput").ap()

    with tile.TileContext(nc) as tc:
        with tc.tile_pool(name="const", bufs=1) as cpool, \
             tc.tile_pool(name="xg", bufs=2) as xpool, \
             tc.tile_pool(name="work", bufs=3) as wpool, \
             tc.tile_pool(name="feat", bufs=1) as fpool, \
             tc.tile_pool(name="outp", bufs=4) as opool, \
             tc.tile_pool(name="psum", bufs=6, space="PSUM") as pspool:

            wfall = cpool.tile([128, NF * DT * O], F16, name="wfall", tag="wfall")
            biasr = cpool.tile([128, O], F32, name="biasr_t", tag="biasr")

            # --- controlled DMA issue order on the sync queue ---
            # interleave group-0 sub-block x loads with weight chunks so the
            # first matmuls (silu feature, then g0..) have operands earliest.
            def xdma(eng, xg, t0, glen):
                eng.dma_start(
                    xg[:].rearrange("p (dt t) -> p dt t", dt=DT),
                    xP_d[:, :, t0:t0 + glen])

            # all x loads on the sync queue, interleaved with the weight
            # chunks in consumption order (the dep-free sync stream keeps
            # its emission order; DMA engines drain it FIFO per engine)
            xg_tiles = {}
            for t0, glen in RANGES:
                tag = f"xg_s{t0}" if glen <= 256 else f"xg_f{t0}"
                xg_tiles[t0] = xpool.tile([128, DT * glen], F16,
                                          name=f"xg{t0}", tag=tag, bufs=1)

            def wfdma(f):
                nc.sync.dma_start(
                    wfall[:, f * DT * O:(f + 1) * DT * O],
                    wf_d[:, f * DT * O:(f + 1) * DT * O])

            # priority order: x before the weight chunks that can wait
            xdma(nc.sync, xg_tiles[0], 0, 128)
            wfdma(0)
            xdma(nc.sync, xg_tiles[128], 128, 128)
            wfdma(1)
            xdma(nc.sync, xg_tiles[256], 256, 256)
            wfdma(2)
            xdma(nc.sync, xg_tiles[512], 512, 512)
            wfdma(3)
            wfdma(4)
            xdma(nc.sync, xg_tiles[1024], 1024, 512)
            wfdma(5)
            wfdma(6)
            xdma(nc.sync, xg_tiles[1536], 1536, 512)
            nc.sync.dma_start(biasr[:], br_d[:])
            biasc = cpool.tile([128, DT], F32, name="biasc_t", tag="biasc")
            nc.sync.dma_start(biasc[:], bc_d[:])
            wft = [wfall[:, i * O:(i + 1) * O] for i in range(NF * DT)]

            # HAM warmup: one accumulation group of full-width matmuls keeps
            # the PE continuously busy so the clock gate opens early; also
            # preload the exp/tanh ACT table set off the critical path.
            wdum = cpool.tile([128, O], F16, name="wdum", tag="wdum")
            nc.vector.memset(wdum[:], 0.0)
            wdum2 = cpool.tile([128, 16], F16, name="wdum2", tag="wdum2")
            nc.scalar.activation(wdum2[:], wdum[:, 0:16], AF.Tanh, scale=0.5)
            wps = pspool.tile([128, O], F32, name="wps", tag="wps", bufs=1)
            for _w in range(8):
                nc.tensor.matmul(wps[:], wdum[:, 0:128], wdum[:],
                                 start=(_w == 0), stop=(_w == 7))

            pending_sub = []    # (psum, row)  token-major head outputs
            pending_full = []   # (psum, ob, t0) o-major tail outputs

            def flush_sub(keep=0):
                while len(pending_sub) > keep:
                    ps, row = pending_sub.pop(0)
                    ot = opool.tile([128, O], F32, name="ot", tag="ot")
                    nc.vector.scalar_tensor_tensor(ot[:], ps[:], 1.0, biasr[:],
                                                   ALU.mult, ALU.add)
                    nc.gpsimd.dma_start(y_d[row:row + 128, :], ot[:])

            def flush_full():
                for ps, ob, t0g in pending_full:
                    ot2 = opool.tile([128, 512], F32, name="ot2", tag="ot2")
                    nc.scalar.activation(ot2[:], ps[:], AF.Identity,
                                         bias=biasc[:, ob:ob + 1])
                    nc.gpsimd.dma_start(
                        y2_d[ob * 128:(ob + 1) * 128,
                             t0g - 512:t0g - 512 + 512], ot2[:])
                pending_full.clear()

            ths = {}

            def emit_tanh(idx):
                t0, glen = RANGES[idx]
                FD = DT * glen
                th = wpool.tile([128, FD], F16, name=f"th{t0}", tag="th", bufs=2)
                nc.scalar.activation(th[:], xg_tiles[t0][:], AF.Tanh, scale=0.5)
                ths[idx] = th

            def emit_headv(idx):
                t0, glen = RANGES[idx]
                FD = DT * glen
                xg = xg_tiles[t0]
                xc = wpool.tile([128, FD], F16, name=f"xc{t0}", tag="xc", bufs=2)
                nc.vector.tensor_scalar(xc[:], xg[:], 3.0, -3.0, ALU.min, ALU.max)
                xh = wpool.tile([128, FD], F16, name=f"xh{t0}", tag="xh", bufs=2)
                nc.vector.tensor_scalar(xh[:], xg[:], 0.5, 0.0, ALU.mult, ALU.add)
                th1 = wpool.tile([128, FD], F16, name=f"th1{t0}", tag="th1",
                                 bufs=2)
                nc.vector.tensor_scalar(th1[:], ths[idx][:], 1.0, 0.0,
                                        ALU.add, ALU.bypass)
                sil = fpool.tile([128, FD], F16, name=f"sil{t0}", tag="sil", bufs=2)
                # silu(x) = (x/2) * (1 + tanh(x/2))   (exact)
                nc.vector.tensor_mul(sil[:], th1[:], xh[:])
                return xc, sil

            for idx, (t0, glen) in enumerate(RANGES):
                FD = DT * glen
                if idx not in ths:
                    emit_tanh(idx)
                # hoist only the next range's tanh (scalar op) ahead of this
                # range's exp chain; its vector ops stay with their range so
                # the in-order vector queue is never blocked on a later DMA
                if idx + 1 < len(RANGES) and (idx + 1) not in ths:
                    emit_tanh(idx + 1)
                xc, sil = emit_headv(idx)
                feats = [sil]
                for i in range(6):
                    z = wpool.tile([128, FD], F16, name=f"z{t0}_{i}", tag="z",
                                   bufs=2)
                    nc.vector.tensor_scalar(z[:], xc[:], XC[i], 0.0,
                                            ALU.subtract, ALU.add)
                    s = wpool.tile([128, FD], F16, name=f"s{t0}_{i}", tag="s",
                                   bufs=2)
                    nc.vector.tensor_mul(s[:], z[:], z[:])
                    g = fpool.tile([128, FD], F16, name=f"g{t0}_{i}",
                                   tag=f"g{i}", bufs=2)
                    nc.scalar.activation(g[:], s[:], AF.Exp, scale=ESC[i])
                    feats.append(g)

                # previous block's PSUM evacuation comes after this block's
                # feature ops in the respective in-order engine queue
                n_mm = NF * DT
                if glen <= 256:
                    flush_sub(keep=1)
                    # token-stationary: lhsT = 128-token feature chunk,
                    # stream the 512-wide weight tiles
                    for c in range(glen // 128):
                        ps = pspool.tile([128, O], F32, name="ps", tag="ps",
                                         bufs=2)
                        i = 0
                        for f in feats:
                            for t_ in range(DT):
                                sl = f[:, t_ * glen + c * 128:
                                       t_ * glen + (c + 1) * 128]
                                nc.tensor.matmul(ps[:], sl, wft[i],
                                                 start=(i == 0),
                                                 stop=(i == n_mm - 1))
                                i += 1
                        pending_sub.append((ps, t0 + c * 128))
                else:
                    flush_sub()
                    flush_full()
                    # weight-stationary: stream 512 tokens per matmul,
                    # psum holds [128 o, 512 tok]; feature-outer order so
                    # each feature is consumed as soon as it is produced
                    pss = [pspool.tile([128, 512], F32, name="ps2",
                                       tag="ps2", bufs=5) for _ in range(4)]
                    if t0 == RANGES[-1][0]:
                        # last group: ob-outer so the four output blocks
                        # finish staggered and evacuation overlaps matmuls
                        # (all features are long since ready here)
                        for ob in range(4):
                            i = 0
                            for fi in range(NF):
                                for t_ in range(DT):
                                    nc.tensor.matmul(
                                        pss[ob][:],
                                        wft[fi * DT + t_][:, ob * 128:(ob + 1) * 128],
                                        feats[fi][:, t_ * glen:(t_ + 1) * glen],
                                        start=(i == 0), stop=(i == n_mm - 1))
                                    i += 1
                            pending_full.append((pss[ob], ob, t0))
                        flush_full()
                    else:
                        for fi in range(NF):
                            for t_ in range(DT):
                                for ob in range(4):
                                    nc.tensor.matmul(
                                        pss[ob][:],
                                        wft[fi * DT + t_][:, ob * 128:(ob + 1) * 128],
                                        feats[fi][:, t_ * glen:(t_ + 1) * glen],
                                        start=(fi == 0 and t_ == 0),
                                        stop=(fi == NF - 1 and t_ == DT - 1))
                        for ob in range(4):
                            pending_full.append((pss[ob], ob, t0))
            flush_sub()
            flush_full()
    nc.compile()
    return nc


def _host_tables(coef, scale_base, scale_sp, bias):
    W = (scale_sp[..., None] * coef).astype(np.float64)        # (O, D, 6)
    V = np.einsum("odj,fj->odf", W, FIT_C)                     # (O, D, 8)
    bias_eff = (bias.astype(np.float64) + V[:, :, 0].sum(1))   # const channel
    V = V[:, :, 1:]                                            # (O, D, NF)
    V[:, :, 0] += scale_base.astype(np.float64)                # silu channel
    # weight tiles: (f, dt) -> [128 d, O] fp16
    wfs = np.empty((NF * DT, 128, O), np.float16)
    for f in range(NF):
        for t in range(DT):
            wfs[f * DT + t] = V[:, t * 128:(t + 1) * 128, f].T
    wf = np.ascontiguousarray(wfs.transpose(1, 0, 2).reshape(128, NF * DT * O))
    biasrep = np.tile(bias_eff.astype(np.float32), (128, 1))
    biascol = np.ascontiguousarray(
        bias_eff.astype(np.float32).reshape(DT, 128).T)
    return wf, np.ascontiguousarray(biasrep), biascol


def kernel(x, coef, scale_base, scale_sp, bias, _trace=False):
    global last_results
    x = np.asarray(x, np.float32)
    coef = np.asarray(coef, np.float32)
    scale_base = np.asarray(scale_base, np.float32)
    scale_sp = np.asarray(scale_sp, np.float32)
    bias = np.asarray(bias, np.float32)
    B, S, Din = x.shape
    assert (B * S, Din) == (N_CORES * TOK, D), (x.shape,)

    if "nc" not in _prog_cache:
        _prog_cache["nc"] = _build_program()
    nc = _prog_cache["nc"]

    wf, biasrep, biascol = _host_tables(coef, scale_base, scale_sp, bias)
    xflat = x.reshape(N_CORES, TOK, D)
    in_maps = []
    for c in range(N_CORES):
        # xP[p, dt, t] = x[t, dt*128+p]
        xP = np.ascontiguousarray(
            xflat[c].T.reshape(DT, 128, TOK).transpose(1, 0, 2)
        ).astype(np.float16)
        in_maps.append({
            "xP": xP,
            "wf": wf, "biasrep": biasrep, "biascol": biascol,
        })
    kw = {}
    if _trace:
        kw.update(trace=True)
    last_results = run_bass_kernel_spmd(nc, in_maps,
                                        core_ids=list(range(N_CORES)), **kw)
    y = np.empty((N_CORES, TOK, O), np.float32)
    for c in range(N_CORES):
        r = last_results.results[c]
        y[c, :512] = r["y"]
        y[c, 512:] = r["y2"].T
    return y.reshape(B, S, O).astype(np.float32)


# revision 36
# speedup vs baseline: 1.0302x; 1.0061x over previous
"""KAN layer (cubic B-spline, uniform grid) for 8 Trainium2 NeuronCores.

Approach: the six cubic B-spline basis functions basis_j(x) = N(1.5x+4.5-j)
are fixed smooth functions of the single scalar x.  They are represented
(to ~1e-3 weighted RMS, fitted offline against the exact Cox-de-Boor
recursion under the N(0,1) input distribution) in the span of seven cheap
feature maps:
    {1, silu(x), exp(-a_i (xc - m_i)^2) i=0..5},  xc = clamp(x, -3, 3)
The constant channel folds into the output bias; the remaining 7 features
feed a fp16 matmul with host-combined weights
    V[o,d,f] = sum_j scale_sp[o,d] * coef[o,d,j] * C[f,j],
so per token the kernel computes  y = feats @ V^T + bias  with contraction
K = 512*7 (vs 512*13 for a truncated-power-basis formulation).

On-chip per 512-token group: clamp + half-scale (vector), tanh + 6 exp
(scalar engine; silu(x) = (x/2)(1+tanh(x/2)) exactly, keeping every scalar
op in the exp/tanh activation-table set -> one ACT table load), 6 squared
distances (vector tensor_tensor, 2x mode), 29 matmuls per 128-token chunk.
The tensor engine is the bottleneck at ~24us/group.

Schedule details: all input DMAs are issued from the sync queue in an
explicit priority order (x blocks interleaved with weight chunks in
consumption order) because the per-engine DMA FIFOs drain in arrival
order.  The first 512 tokens are processed in small sub-groups with
token-stationary matmuls (psum [tok, o], vector-engine bias+evacuation)
so matmuls start ~10us in and keep the PE clock gate (HAM) open; the
remaining 1536 tokens use weight-stationary matmuls (psum [o, tok],
scalar-engine per-partition-bias evacuation, o-major y2 output that the
host transposes).  Each next range's clamp/tanh/silu head is hoisted
ahead of the current range's exp chain on the in-order scalar queue, and
PSUM evacuation for range r is emitted after range r+1's feature ops, so
neither the vector nor scalar queue ever stalls the next range's
features.  The last group runs o-block-outer so its evacuation overlaps
the matmul stream.

Data parallel over tokens: core c processes batch row c (2048 tokens).
"""

import numpy as np

import concourse.bass as bass
import concourse.mybir as mybir
import concourse.tile as tile
from concourse import bacc
from concourse.bass_utils import run_bass_kernel_spmd

F32 = mybir.dt.float32
F16 = mybir.dt.float16
ALU = mybir.AluOpType
AF = mybir.ActivationFunctionType

N_CORES = 8
D = 512          # in_features
O = 512          # out_features
TOK = 2048       # tokens per core
DT = D // 128    # d-tiles
NF = 7           # matmul features: silu + 6 gaussians

# offline fit of basis_j(u) ~ C[0,j] + C[1,j]*silu(x) + sum_i C[2+i,j]*g_i(u)
# with g_i = exp(-W_i (u - M_i)^2), u = 1.5x + 4.5 clamped to [0,9].
GAUSS_M = [2.0136448196366565, 3.0020084036623915, 4.0003483932813335,
           4.999588735443863, 5.997713911954367, 6.985305153374808]
GAUSS_W = [1.2619403072231266, 1.1976479467083618, 1.1876576172436166,
           1.1873248687663445, 1.1985693610575836, 1.2670653033973047]
FIT_C = np.array([
 [-0.014076312408546078, 0.0030093286289790723, -0.0015483720228174193, 0.001051762014029704, -0.0004897521315850754, -0.000887221022976455],
 [0.005876740626533684, -0.0015677343525573972, 0.0011509219211435948, -0.0011444952851484866, 0.0014410957474447201, -0.005142005556391447],
 [0.6883806333564002, -0.04083848226036093, 0.008060321332289084, -0.0037685760483973907, 0.0020933252562422002, -0.0007681411586616727],
 [-0.022738777814332027, 0.6876227273109307, -0.04291372848231555, 0.007127102690404596, -0.0022787476226903756, 0.0003672673217971428],
 [0.014126054568335171, -0.044924255427300344, 0.6933655085603457, -0.045792817865033045, 0.007933603676615806, -0.0015453651987665717],
 [0.006650677872101163, 0.005744884186467451, -0.044029610498844615, 0.6917600947426843, -0.04263220709635316, 0.005718618412934847],
 [0.0067666440225774855, -0.003981106455691672, 0.00836428843888494, -0.04422963468506508, 0.6892332271718113, -0.029107509213147646],
 [0.0036261271724442106, 0.0009189834250657185, -0.0028350790762476097, 0.007121213292172948, -0.03905988194390376, 0.6828562285636222],
], dtype=np.float64)   # rows: const, silu, g0..g5; cols: basis j

# gaussian centers in x-space and exp scales: g = exp(ESC_i * (xc - XC_i)^2)
XC = [(m - 4.5) / 1.5 for m in GAUSS_M]
ESC = [-2.25 * w for w in GAUSS_W]

# token ranges: first 512 tokens split fine so matmuls start early
RANGES = [(0, 128), (128, 128), (256, 256),
          (512, 512), (1024, 512), (1536, 512)]

_prog_cache = {}
last_results = None  # BassKernelResults of the most recent run (for test.py)


def _build_program():
    nc = bacc.Bacc("TRN2", target_bir_lowering=False, debug=False,
                   num_devices=N_CORES)
    # x pre-packed on host: xP[p, dt, t] = x[t, dt*128+p], fp16
    xP_d = nc.dram_tensor("xP", [128, DT, TOK], F16, kind="ExternalInput").ap()
    wf_d = nc.dram_tensor("wf", [128, NF * DT * O], F16, kind="ExternalInput").ap()
    br_d = nc.dram_tensor("biasrep", [128, O], F32, kind="ExternalInput").ap()
    bc_d = nc.dram_tensor("biascol", [128, DT], F32, kind="ExternalInput").ap()
    # first 512 tokens come out token-major, the rest o-major (host stitches)
    y_d = nc.dram_tensor("y", [512, O], F32, kind="ExternalOutput").ap()
    y2_d = nc.dram_tensor("y2", [O, TOK - 512], F32, kind="ExternalOutput").ap()

    with tile.TileContext(nc) as tc:
        with tc.tile_pool(name="const", bufs=1) as cpool, \
             tc.tile_pool(name="xg", bufs=2) as xpool, \
             tc.tile_pool(name="work", bufs=3) as wpool, \
             tc.tile_pool(name="feat", bufs=1) as fpool, \
             tc.tile_pool(name="outp", bufs=4) as opool, \
             tc.tile_pool(name="psum", bufs=6, space="PSUM") as pspool:

            wfall = cpool.tile([128, NF * DT * O], F16, name="wfall", tag="wfall")
            biasr = cpool.tile([128, O], F32, name="biasr_t", tag="biasr")

            # --- controlled DMA issue order on the sync queue ---
            # interleave group-0 sub-block x loads with weight chunks so the
            # first matmuls (silu feature, then g0..) have operands earliest.
            def xdma(eng, xg, t0, glen):
                eng.dma_start(
                    xg[:].rearrange("p (dt t) -> p dt t", dt=DT),
                    xP_d[:, :, t0:t0 + glen])

            # all x loads on the sync queue, interleaved with the weight
            # chunks in consumption order (the dep-free sync stream keeps
            # its emission order; DMA engines drain it FIFO per engine)
            xg_tiles = {}
            for t0, glen in RANGES:
                tag = f"xg_s{t0}" if glen <= 256 else f"xg_f{t0}"
                xg_tiles[t0] = xpool.tile([128, DT * glen], F16,
                                          name=f"xg{t0}", tag=tag, bufs=1)

            def wfdma(f):
                nc.sync.dma_start(
                    wfall[:, f * DT * O:(f + 1) * DT * O],
                    wf_d[:, f * DT * O:(f + 1) * DT * O])

            # priority order: x before the weight chunks that can wait
            xdma(nc.sync, xg_tiles[0], 0, 128)
            wfdma(0)
            xdma(nc.sync, xg_tiles[128], 128, 128)
            wfdma(1)
            xdma(nc.sync, xg_tiles[256], 256, 256)
            wfdma(2)
            xdma(nc.sync, xg_tiles[512], 512, 512)
            wfdma(3)
            wfdma(4)
            xdma(nc.sync, xg_tiles[1024], 1024, 512)
            wfdma(5)
            wfdma(6)
            xdma(nc.sync, xg_tiles[1536], 1536, 512)
            nc.sync.dma_start(biasr[:], br_d[:])
            biasc = cpool.tile([128, DT], F32, name="biasc_t", tag="biasc")
            nc.sync.dma_start(biasc[:], bc_d[:])
            wft = [wfall[:, i * O:(i + 1) * O] for i in range(NF * DT)]

            # HAM warmup: one accumulation group of full-width matmuls keeps
            # the PE continuously busy so the clock gate opens early; also
            # preload the exp/tanh ACT table set off the critical path.
            wdum = cpool.tile([128, O], F16, name="wdum", tag="wdum")
            nc.vector.memset(wdum[:], 0.0)
            wdum2 = cpool.tile([128, 16], F16, name="wdum2", tag="wdum2")
            nc.scalar.activation(wdum2[:], wdum[:, 0:16], AF.Tanh, scale=0.5)
            wps = pspool.tile([128, O], F32, name="wps", tag="wps", bufs=1)
            for _w in range(16):
                nc.tensor.matmul(wps[:], wdum[:, 0:128], wdum[:],
                                 start=(_w == 0), stop=(_w == 15))

            pending_sub = []    # (psum, row)  token-major head outputs
            pending_full = []   # (psum, ob, t0) o-major tail outputs

            def flush_sub(keep=0):
                while len(pending_sub) > keep:
                    ps, row = pending_sub.pop(0)
                    ot = opool.tile([128, O], F32, name="ot", tag="ot")
                    nc.vector.scalar_tensor_tensor(ot[:], ps[:], 1.0, biasr[:],
                                                   ALU.mult, ALU.add)
                    nc.gpsimd.dma_start(y_d[row:row + 128, :], ot[:])

            def flush_full():
                for ps, ob, t0g in pending_full:
                    ot2 = opool.tile([128, 512], F32, name="ot2", tag="ot2")
                    nc.scalar.activation(ot2[:], ps[:], AF.Identity,
                                         bias=biasc[:, ob:ob + 1])
                    nc.gpsimd.dma_start(
                        y2_d[ob * 128:(ob + 1) * 128,
                             t0g - 512:t0g - 512 + 512], ot2[:])
                pending_full.clear()

            heads = {}

            def emit_head(idx):
                t0, glen = RANGES[idx]
                FD = DT * glen
                xg = xg_tiles[t0]
                xc = wpool.tile([128, FD], F16, name=f"xc{t0}", tag="xc", bufs=2)
                nc.vector.tensor_scalar(xc[:], xg[:], 3.0, -3.0, ALU.min, ALU.max)
                xh = wpool.tile([128, FD], F16, name=f"xh{t0}", tag="xh", bufs=2)
                nc.vector.tensor_scalar(xh[:], xg[:], 0.5, 0.0, ALU.mult, ALU.add)
                th = wpool.tile([128, FD], F16, name=f"th{t0}", tag="th", bufs=2)
                nc.scalar.activation(th[:], xg[:], AF.Tanh, scale=0.5)
                th1 = wpool.tile([128, FD], F16, name=f"th1{t0}", tag="th1",
                                 bufs=2)
                nc.vector.tensor_scalar(th1[:], th[:], 1.0, 0.0,
                                        ALU.add, ALU.bypass)
                sil = fpool.tile([128, FD], F16, name=f"sil{t0}", tag="sil", bufs=2)
                # silu(x) = (x/2) * (1 + tanh(x/2))   (exact)
                nc.vector.tensor_mul(sil[:], th1[:], xh[:])
                heads[idx] = (xc, sil)

            for idx, (t0, glen) in enumerate(RANGES):
                FD = DT * glen
                if idx not in heads:
                    emit_head(idx)
                # hoist the next range's head (its tanh) ahead of this
                # range's exp chain on the in-order scalar queue
                if idx + 1 < len(RANGES) and (idx + 1) not in heads:
                    emit_head(idx + 1)
                xc, sil = heads[idx]
                feats = [sil]
                for i in range(6):
                    z = wpool.tile([128, FD], F16, name=f"z{t0}_{i}", tag="z",
                                   bufs=2)
                    nc.vector.tensor_scalar(z[:], xc[:], XC[i], 0.0,
                                            ALU.subtract, ALU.add)
                    s = wpool.tile([128, FD], F16, name=f"s{t0}_{i}", tag="s",
                                   bufs=2)
                    nc.vector.tensor_mul(s[:], z[:], z[:])
                    g = fpool.tile([128, FD], F16, name=f"g{t0}_{i}",
                                   tag=f"g{i}", bufs=2)
                    nc.scalar.activation(g[:], s[:], AF.Exp, scale=ESC[i])
                    feats.append(g)

                # previous block's PSUM evacuation comes after this block's
                # feature ops in the respective in-order engine queue
                n_mm = NF * DT
                if glen <= 256:
                    flush_sub(keep=1)
                    # token-stationary: lhsT = 128-token feature chunk,
                    # stream the 512-wide weight tiles
                    for c in range(glen // 128):
                        ps = pspool.tile([128, O], F32, name="ps", tag="ps",
                                         bufs=2)
                        i = 0
                        for f in feats:
                            for t_ in range(DT):
                                sl = f[:, t_ * glen + c * 128:
                                       t_ * glen + (c + 1) * 128]
                                nc.tensor.matmul(ps[:], sl, wft[i],
                                                 start=(i == 0),
                                                 stop=(i == n_mm - 1))
                                i += 1
                        pending_sub.append((ps, t0 + c * 128))
                else:
                    flush_sub()
                    flush_full()
                    # weight-stationary: stream 512 tokens per matmul,
                    # psum holds [128 o, 512 tok]; feature-outer order so
                    # each feature is consumed as soon as it is produced
                    pss = [pspool.tile([128, 512], F32, name="ps2",
                                       tag="ps2", bufs=5) for _ in range(4)]
                    if t0 == RANGES[-1][0]:
                        # last group: ob-outer so the four output blocks
                        # finish staggered and evacuation overlaps matmuls
                        # (all features are long since ready here)
                        for ob in range(4):
                            i = 0
                            for fi in range(NF):
                                for t_ in range(DT):
                                    nc.tensor.matmul(
                                        pss[ob][:],
                                        wft[fi * DT + t_][:, ob * 128:(ob + 1) * 128],
                                        feats[fi][:, t_ * glen:(t_ + 1) * glen],
                                        start=(i == 0), stop=(i == n_mm - 1))
                                    i += 1
                            pending_full.append((pss[ob], ob, t0))
                        flush_full()
                    else:
                        for fi in range(NF):
                            for t_ in range(DT):
                                for ob in range(4):
                                    nc.tensor.matmul(
                                        pss[ob][:],
                                        wft[fi * DT + t_][:, ob * 128:(ob + 1) * 128],
                                        feats[fi][:, t_ * glen:(t_ + 1) * glen],
                                        start=(fi == 0 and t_ == 0),
                                        stop=(fi == NF - 1 and t_ == DT - 1))
                        for ob in range(4):
                            pending_full.append((pss[ob], ob, t0))
            flush_sub()
            flush_full()
    nc.compile()
    return nc


def _host_tables(coef, scale_base, scale_sp, bias):
    W = (scale_sp[..., None] * coef).astype(np.float64)        # (O, D, 6)
    V = np.einsum("odj,fj->odf", W, FIT_C)                     # (O, D, 8)
    bias_eff = (bias.astype(np.float64) + V[:, :, 0].sum(1))   # const channel
    V = V[:, :, 1:]                                            # (O, D, NF)
    V[:, :, 0] += scale_base.astype(np.float64)                # silu channel
    # weight tiles: (f, dt) -> [128 d, O] fp16
    wfs = np.empty((NF * DT, 128, O), np.float16)
    for f in range(NF):
        for t in range(DT):
            wfs[f * DT + t] = V[:, t * 128:(t + 1) * 128, f].T
    wf = np.ascontiguousarray(wfs.transpose(1, 0, 2).reshape(128, NF * DT * O))
    biasrep = np.tile(bias_eff.astype(np.float32), (128, 1))
    biascol = np.ascontiguousarray(
        bias_eff.astype(np.float32).reshape(DT, 128).T)
    return wf, np.ascontiguousarray(biasrep), biascol


def kernel(x, coef, scale_base, scale_sp, bias, _trace=False):
    global last_results
    x = np.asarray(x, np.float32)
    coef = np.asarray(coef, np.float32)
    scale_base = np.asarray(scale_base, np.float32)
    scale_sp = np.asarray(scale_sp, np.float32)
    bias = np.asarray(bias, np.float32)
    B, S, Din = x.shape
    assert (B * S, Din) == (N_CORES * TOK, D), (x.shape,)

    if "nc" not in _prog_cache:
        _prog_cache["nc"] = _build_program()
    nc = _prog_cache["nc"]

    wf, biasrep, biascol = _host_tables(coef, scale_base, scale_sp, bias)
    xflat = x.reshape(N_CORES, TOK, D)
    in_maps = []
    for c in range(N_CORES):
        # xP[p, dt, t] = x[t, dt*128+p]
        xP = np.ascontiguousarray(
            xflat[c].T.reshape(DT, 128, TOK).transpose(1, 0, 2)
        ).astype(np.float16)
        in_maps.append({
            "xP": xP,
            "wf": wf, "biasrep": biasrep, "biascol": biascol,
        })
    kw = {}
    if _trace:
        kw.update(trace=True)
    last_results = run_bass_kernel_spmd(nc, in_maps,
                                        core_ids=list(range(N_CORES)), **kw)
    y = np.empty((N_CORES, TOK, O), np.float32)
    for c in range(N_CORES):
        r = last_results.results[c]
        y[c, :512] = r["y"]
        y[c, 512:] = r["y2"].T
    return y.reshape(B, S, O).astype(np.float32)
